# revision 13
# baseline (speedup 1.0000x reference)
"""Trainium2 Bass kernel for the CNN-VAE loss:

    prob = einsum('klb,hwb->klhw', beta, A) * 5000
    mse  = mean(sum(|x - prob[:, :, None]|^2, axis=1))

Strategy (v5: algebraic expansion, bf16 stream)
-----------------------------------------------
Expand  sum |x - p|^2 = sum x^2 - 2*sum x*p + C*sum p^2  (p broadcast over
the C=3 channel dim).  With p = SCALE * einsum('klb,hwb', beta, A):

  T1 = sum x^2            -> device; split between ACT Square+accum_out and
       DVE fused scalar_tensor_tensor (x*x with accum_out) so both engines
       stay inside the per-group DMA window.
  T2 = -2*SCALE * sum_b sum_pix A[pix,b]*Y[b,pix],
       Y[b,pix] = sum_{kl,c} beta[kl,b]*x[kl,c,pix]
       -> Y on the PE: beta (128x3 bf16) stationary, x streaming, the c-sum
       folded into the PSUM accumulation (3 matmuls per 500-px chunk).  The
       A-weighted pixel reduction is one fused DVE scalar_tensor_tensor
       straight out of PSUM per chunk.
  T3 = C*SCALE^2 * sum_kl beta^T (A^T A) beta  -> host, f64, from the tiny
       beta/A inputs.

x streams as bf16 (halves HBM traffic; quantization contributes <1e-9
relative error vs the 2e-2 tolerance since T1+T2 are ~1e-7 of the total).
The hw axis is sharded across 8 cores (5000 px each).  Startup tricks:
beta rides inside group 0's x DMA (a separate descriptor queue lands ~5us
late), the ACT Square spline table and the PE HAM clock gate are warmed
with dummy work while the first DMA is in flight, and all group buffers
are resident simultaneously so every x DMA issues back-to-back.
"""

import numpy as np

K, L, NB, H, W = 16, 8, 3, 200, 200
KL = K * L          # 128 partitions
C = 3               # broadcast channel dim of x
HW = H * W          # 40000
N_CORES = 8
HW_SHARD = HW // N_CORES   # 5000
GROUP = 1000               # pixels per steady-state iteration
NG = HW_SHARD // GROUP     # 5
HALF = GROUP // 2          # 500: matmul free-dim chunk (fits one PSUM bank)
BANK = 512                 # PSUM bank width in f32
XCOLS = C * GROUP          # 3000 x columns per group
GW = XCOLS + 8             # group row width: x + embedded beta (3) + pad
DVE_SQ = 888               # x^2 columns handled by DVE (rest on ACT)
SCALE = 5000.0
DENOM = float(K * C * H * W)  # mean denominator (sum over L folded in)

_NC = None


def _build():
    global _NC
    if _NC is not None:
        return _NC
    from contextlib import ExitStack

    import concourse.bacc as bacc
    import concourse.mybir as mybir
    import concourse.tile as tile

    f32 = mybir.dt.float32
    bf16 = mybir.dt.bfloat16
    nc = bacc.Bacc("TRN2", target_bir_lowering=False, debug=False)

    f8 = mybir.dt.float8e4
    xg = nc.dram_tensor("xg", [NG, KL, GW], f8, kind="ExternalInput").ap()
    asb = nc.dram_tensor("asb", [NB, NG, 2, BANK], bf16, kind="ExternalInput").ap()
    out = nc.dram_tensor("out", [1, 20], f32, kind="ExternalOutput").ap()

    with tile.TileContext(nc) as tc, ExitStack() as ctx:
        const = ctx.enter_context(tc.tile_pool(name="const", bufs=1))
        xpool = ctx.enter_context(tc.tile_pool(name="x", bufs=NG))
        spool = ctx.enter_context(tc.tile_pool(name="scr", bufs=2))
        ppool = ctx.enter_context(tc.tile_pool(name="psum", bufs=3, space="PSUM"))
        wpool = ctx.enter_context(tc.tile_pool(name="wps", bufs=1, space="PSUM"))

        # A^T on the GPSIMD SWDGE queue (not needed until ~13us; lands ~12)
        a_sb = const.tile([NB, NG, 2, BANK], bf16)
        nc.gpsimd.dma_start(a_sb[:], asb[:])
        ones = const.tile([KL, 1], f32)
        nc.vector.memset(ones[:], 0.0)
        nc.vector.tensor_scalar_add(ones[:], ones[:], 1.0)

        acc = const.tile([KL, 20], f32)
        nc.vector.memset(acc[:], 0.0)

        # warm the ACT Square spline table while DMAs are in flight
        warm = const.tile([KL, 8], f32)
        nc.vector.memset(warm[:], 0.0)
        nc.scalar.activation(warm[:], warm[:], mybir.ActivationFunctionType.Square)

        # warm the PE HAM clock gate (idle default is half clock; ~3.4us of
        # activity lifts it before the real matmuls arrive)
        wmm = const.tile([KL, BANK], bf16)
        nc.vector.memset(wmm[:], 0.0)
        ydum = wpool.tile([NB, BANK], f32)
        for _ in range(12):
            nc.tensor.matmul(
                ydum[:, :BANK], wmm[:, :NB], wmm[:], start=True, stop=True
            )

        xts = []
        for g in range(NG):
            xt = xpool.tile([KL, GW], f8)
            nc.sync.dma_start(xt[:], xg[g])
            xts.append(xt)
        b_sb = xts[0][:, XCOLS : XCOLS + NB]  # beta rides in group 0

        for g in range(NG):
            xt = xts[g]

            # PE: Y[b, pix] += sum_kl beta[kl,b]*x[kl,c,pix], c folded into
            # the PSUM accumulation group
            yt = ppool.tile([NB, 2, BANK], f32)
            for h in range(2):
                for c in range(C):
                    nc.tensor.matmul(
                        yt[:, h, :HALF],
                        b_sb,
                        xt[:, c * GROUP + h * HALF : c * GROUP + (h + 1) * HALF],
                        start=(c == 0),
                        stop=(c == C - 1),
                    )

            def emit_t2(g=g, xt=xt, yt=yt):
                t2s = spool.tile([NB, 2, HALF], bf16)
                nc.vector.scalar_tensor_tensor(
                    out=t2s[:],
                    in0=yt[:, :, :HALF],
                    scalar=1.0,
                    in1=a_sb[:, g, :, :HALF],
                    op0=mybir.AluOpType.mult,
                    op1=mybir.AluOpType.mult,
                    accum_out=acc[0:NB, 10 + g : 11 + g],
                )

            def emit_sq(g=g, xt=xt):
                sqs = spool.tile([KL, DVE_SQ], bf16)
                nc.vector.scalar_tensor_tensor(
                    out=sqs[:],
                    in0=xt[:, :DVE_SQ],
                    scalar=1.0,
                    in1=xt[:, :DVE_SQ],
                    op0=mybir.AluOpType.mult,
                    op1=mybir.AluOpType.mult,
                    accum_out=acc[:, 5 + g : 6 + g],
                )
                sqa = spool.tile([KL, XCOLS - DVE_SQ], bf16)
                nc.scalar.activation(
                    sqa[:],
                    xt[:, DVE_SQ:XCOLS],
                    mybir.ActivationFunctionType.Square,
                    accum_out=acc[:, g : g + 1],
                )

            if g == NG - 1:
                emit_sq()  # tail: squares don't depend on the matmuls
                emit_t2()
            else:
                emit_t2()
                emit_sq()

            if g == 1:
                # probe: custom-DVE affine_mul_reduce col rate (bf16, 888)
                pscr = spool.tile([KL, DVE_SQ], bf16)
                psq = spool.tile([KL, DVE_SQ], bf16)
                nc.vector.scalar_tensor_tensor(
                    out=psq[:], in0=xts[1][:, :DVE_SQ], scalar=1.0,
                    in1=xts[1][:, :DVE_SQ], op0=mybir.AluOpType.mult,
                    op1=mybir.AluOpType.mult, accum_out=acc[:, 19:20],
                )
                nc.vector.affine_mul_reduce(
                    out=pscr[:], accum_out=acc[:, 19:20],
                    in0=psq[:], in1=psq[:], scale=1.0, bias=0.0,
                )

        po = wpool.tile([1, 32], f32)
        nc.tensor.matmul(po[:, :20], ones[:], acc[:], start=True, stop=True)
        out_sb = const.tile([1, 20], f32)
        nc.vector.tensor_copy(out_sb[:], po[:, :20])
        nc.sync.dma_start(out[:], out_sb[:])

    nc.compile()
    _NC = nc
    return nc


def _make_in_maps(x, beta, A):
    import ml_dtypes

    bf16 = ml_dtypes.bfloat16
    f8 = ml_dtypes.float8_e4m3
    x = np.asarray(x, dtype=np.float32)
    beta = np.asarray(beta, dtype=np.float32)
    A = np.asarray(A, dtype=np.float32)

    # (KL, C, cores, NG, GROUP) -> (cores, NG, KL, C*GROUP), plus the
    # embedded beta columns in group 0
    xr = x.reshape(KL, C, N_CORES, NG, GROUP).transpose(2, 3, 0, 1, 4)
    xgs = np.zeros((N_CORES, NG, KL, GW), dtype=f8)
    xgs[:, :, :, :XCOLS] = xr.reshape(N_CORES, NG, KL, XCOLS).astype(f8)
    xgs[:, 0, :, XCOLS : XCOLS + NB] = beta.reshape(KL, NB).astype(f8)[None]
    # A^T shards packed to PSUM-bank geometry: (cores, NB, NG, 2, BANK)
    a5 = A.reshape(N_CORES, NG, 2, HALF, NB).transpose(0, 4, 1, 2, 3)
    at = np.zeros((N_CORES, NB, NG, 2, BANK), dtype=bf16)
    at[:, :, :, :, :HALF] = a5.astype(bf16)

    in_maps = []
    for i in range(N_CORES):
        in_maps.append(
            {
                "xg": np.ascontiguousarray(xgs[i]),
                "asb": at[i],
            }
        )
    return in_maps


def _run(in_maps, trace=False, **kwargs):
    from concourse import bass_utils

    nc = _build()
    return bass_utils.run_bass_kernel_spmd(
        nc, in_maps, list(range(N_CORES)), trace=trace, **kwargs
    )


def _combine(results, beta, A):
    t1 = 0.0
    t2 = 0.0
    for r in results:
        o = np.asarray(r["out"], dtype=np.float64)
        t1 += float(np.sum(o[0, :10]))
        t2 += float(np.sum(o[0, 10:15]))
    bf = np.asarray(beta, dtype=np.float64).reshape(KL, NB)
    af = np.asarray(A, dtype=np.float64).reshape(HW, NB)
    m = af.T @ af  # 3x3
    t3 = float(C) * SCALE * SCALE * float(np.einsum("kb,bc,kc->", bf, m, bf))
    total = t1 - 2.0 * SCALE * t2 + t3
    return np.float32(total / DENOM)


def kernel(x, beta, A):
    res = _run(_make_in_maps(x, beta, A))
    return _combine(res.results, beta, A)


# revision 15
# speedup vs baseline: 1.0526x; 1.0526x over previous
"""Trainium2 Bass kernel for the CNN-VAE loss:

    prob = einsum('klb,hwb->klhw', beta, A) * 5000
    mse  = mean(sum(|x - prob[:, :, None]|^2, axis=1))

Strategy (v7: algebraic expansion, fp8 stream)
----------------------------------------------
Expand  sum |x - p|^2 = sum x^2 - 2*sum x*p + C*sum p^2  (p broadcast over
the C=3 channel dim).  With p = SCALE * einsum('klb,hwb', beta, A):

  T1 = sum x^2            -> device; split between ACT Square+accum_out and
       DVE fused scalar_tensor_tensor (x*x with accum_out) so both engines
       stay inside the per-group window.
  T2 = -2*SCALE * sum_b sum_pix A[pix,b]*Y[b,pix],
       Y[b,pix] = sum_{kl,c} beta[kl,b]*x[kl,c,pix]
       -> Y on the PE: beta (128x3) stationary, x streaming, the c-sum
       folded into the PSUM accumulation (3 matmuls per <=512-px chunk).
       The A-weighted pixel reduction is one fused DVE scalar_tensor_tensor
       (3D access pattern over the PSUM banks) per group.
  T3 = C*SCALE^2 * sum_kl beta^T (A^T A) beta  -> host, f64, from the tiny
       beta/A inputs.

x streams as fp8_e4m3 (quarters HBM traffic vs f32; quantization shifts
the result by ~1e-10 relative since T1+T2 are ~1e-7 of the total and T3
is computed exactly).  The hw axis is sharded across 8 cores (5000 px
each).  Latency tricks: beta rides inside group 0's x DMA, the ACT Square
spline table and the PE HAM clock gate are warmed with dummy work during
the DMA preamble, group sizes are staggered (512 / 4x1024 / 392 px) so
the pipeline fills early and drains fast, all group buffers are resident
simultaneously, and the 18 partial accumulators are collapsed to a single
(1,20) row on the PE before one tiny output DMA.
"""

import numpy as np

K, L, NB, H, W = 16, 8, 3, 200, 200
KL = K * L          # 128 partitions
C = 3               # broadcast channel dim of x
HW = H * W          # 40000
N_CORES = 8
HW_SHARD = HW // N_CORES   # 5000
BANK = 512                 # PSUM bank width in f32 (max matmul free dim)
PIX_G = [512, 1024, 1024, 1024, 1024, 392]   # pixels per group
NG = len(PIX_G)
DVE_FRAC = 0.296           # share of x^2 columns on DVE (rest on ACT)
SCALE = 5000.0
DENOM = float(K * C * H * W)  # mean denominator (sum over L folded in)

# derived layout: per-group column offsets in the packed x row
_G_OFF = []
_off = 0
for _g, _p in enumerate(PIX_G):
    _G_OFF.append(_off)
    _off += C * _p + (8 if _g == 0 else 0)  # group 0 carries beta + pad
XROW = _off
# chunk table: (group, start_pixel_within_group, width)
CHUNKS = []
for _g, _p in enumerate(PIX_G):
    _s = 0
    while _s < _p:
        CHUNKS.append((_g, _s, min(BANK, _p - _s)))
        _s += BANK
NCH = len(CHUNKS)          # 10

_NC = None


def _build():
    global _NC
    if _NC is not None:
        return _NC
    from contextlib import ExitStack

    import concourse.bacc as bacc
    import concourse.mybir as mybir
    import concourse.tile as tile

    f32 = mybir.dt.float32
    bf16 = mybir.dt.bfloat16
    f8 = mybir.dt.float8e4
    nc = bacc.Bacc("TRN2", target_bir_lowering=False, debug=False)

    xg = nc.dram_tensor("xg", [KL, XROW], f8, kind="ExternalInput").ap()
    asb = nc.dram_tensor("asb", [NB, NCH, BANK], bf16, kind="ExternalInput").ap()
    out = nc.dram_tensor("out", [1, 20], f32, kind="ExternalOutput").ap()

    with tile.TileContext(nc) as tc, ExitStack() as ctx:
        const = ctx.enter_context(tc.tile_pool(name="const", bufs=1))
        xpool = ctx.enter_context(tc.tile_pool(name="x", bufs=NG))
        spool = ctx.enter_context(tc.tile_pool(name="scr", bufs=2))
        ppool = ctx.enter_context(tc.tile_pool(name="psum", bufs=3, space="PSUM"))
        wpool = ctx.enter_context(tc.tile_pool(name="wps", bufs=1, space="PSUM"))

        # A^T (bank-aligned chunks) on the GPSIMD SWDGE queue
        a_sb = const.tile([NB, NCH, BANK], bf16)
        nc.gpsimd.dma_start(a_sb[:], asb[:])
        ones = const.tile([KL, 1], f32)
        nc.vector.memset(ones[:], 0.0)
        nc.vector.tensor_scalar_add(ones[:], ones[:], 1.0)

        acc = const.tile([KL, 20], f32)
        nc.vector.memset(acc[:], 0.0)

        # warm the ACT Square spline table while DMAs are in flight
        warm = const.tile([KL, 8], f32)
        nc.vector.memset(warm[:], 0.0)
        nc.scalar.activation(warm[:], warm[:], mybir.ActivationFunctionType.Square)

        # warm the PE HAM clock gate (idle default is half clock)
        wmm = const.tile([KL, BANK], bf16)
        nc.vector.memset(wmm[:], 0.0)
        ydum = wpool.tile([NB, BANK], f32)
        for _ in range(9):
            nc.tensor.matmul(
                ydum[:, :BANK], wmm[:, :NB], wmm[:], start=True, stop=True
            )

        XW = max(C * p + (8 if g == 0 else 0) for g, p in enumerate(PIX_G))
        xts = []
        for g, p in enumerate(PIX_G):
            w = C * p + (8 if g == 0 else 0)
            xt = xpool.tile([KL, XW], f8)
            nc.sync.dma_start(xt[:, :w], xg[:, _G_OFF[g] : _G_OFF[g] + w])
            xts.append(xt)
        b_sb = xts[0][:, C * PIX_G[0] : C * PIX_G[0] + NB]  # beta in group 0

        ch_of_g = [[i for i, c in enumerate(CHUNKS) if c[0] == g] for g in range(NG)]

        for g, p in enumerate(PIX_G):
            xt = xts[g]
            nch = len(ch_of_g[g])
            c0 = ch_of_g[g][0]

            # PE: Y[b, pix] += sum_kl beta[kl,b]*x[kl,c,pix]
            yt = ppool.tile([NB, nch, BANK], f32)
            for j in range(nch):
                _, s, wdt = CHUNKS[c0 + j]
                for c in range(C):
                    nc.tensor.matmul(
                        yt[:, j, :wdt],
                        b_sb,
                        xt[:, c * p + s : c * p + s + wdt],
                        start=(c == 0),
                        stop=(c == C - 1),
                    )

            def emit_t2(g=g, yt=yt, nch=nch, c0=c0):
                # chunks within a group share one width (512, or a lone tail)
                wdt = CHUNKS[c0][2]
                t2s = spool.tile([NB, nch, wdt], bf16)
                nc.vector.scalar_tensor_tensor(
                    out=t2s[:],
                    in0=yt[:, :, :wdt],
                    scalar=1.0,
                    in1=a_sb[:, c0 : c0 + nch, :wdt],
                    op0=mybir.AluOpType.mult,
                    op1=mybir.AluOpType.mult,
                    accum_out=acc[0:NB, 2 * NG + g : 2 * NG + g + 1],
                )

            def emit_sq(g=g, xt=xt, p=p):
                dve_cols = (int(C * p * DVE_FRAC) // 4) * 4
                sqs = spool.tile([KL, dve_cols], bf16)
                nc.vector.scalar_tensor_tensor(
                    out=sqs[:],
                    in0=xt[:, :dve_cols],
                    scalar=1.0,
                    in1=xt[:, :dve_cols],
                    op0=mybir.AluOpType.mult,
                    op1=mybir.AluOpType.mult,
                    accum_out=acc[:, NG + g : NG + g + 1],
                )
                sqa = spool.tile([KL, C * p - dve_cols], bf16)
                nc.scalar.activation(
                    sqa[:],
                    xt[:, dve_cols : C * p],
                    mybir.ActivationFunctionType.Square,
                    accum_out=acc[:, g : g + 1],
                )

            if g >= NG - 2:
                emit_sq()  # tail: squares don't depend on the matmuls
                emit_t2()
            else:
                emit_t2()
                emit_sq()

        # collapse the (128,20) partials to one row on the PE, then one DMA
        po = wpool.tile([1, 32], f32)
        nc.tensor.matmul(po[:, :20], ones[:], acc[:], start=True, stop=True)
        out_sb = const.tile([1, 20], f32)
        nc.vector.tensor_copy(out_sb[:], po[:, :20])
        nc.sync.dma_start(out[:], out_sb[:])

    nc.compile()
    _NC = nc
    return nc


def _make_in_maps(x, beta, A):
    import ml_dtypes

    bf16 = ml_dtypes.bfloat16
    f8 = ml_dtypes.float8_e4m3
    x = np.asarray(x, dtype=np.float32)
    beta = np.asarray(beta, dtype=np.float32)
    A = np.asarray(A, dtype=np.float32)

    xr = x.reshape(KL, C, N_CORES, HW_SHARD)     # (128, 3, cores, 5000)
    bt = beta.reshape(KL, NB).astype(f8)
    at_full = A.reshape(N_CORES, HW_SHARD, NB)   # (cores, 5000, 3)

    in_maps = []
    for i in range(N_CORES):
        xs = xr[:, :, i, :]                      # (128, 3, 5000)
        row = np.zeros((KL, XROW), dtype=f8)
        p0 = 0
        for g, p in enumerate(PIX_G):
            blk = xs[:, :, p0 : p0 + p].reshape(KL, C * p).astype(f8)
            row[:, _G_OFF[g] : _G_OFF[g] + C * p] = blk
            p0 += p
        row[:, C * PIX_G[0] : C * PIX_G[0] + NB] = bt
        ash = np.zeros((NB, NCH, BANK), dtype=bf16)
        for j, (g, s, wdt) in enumerate(CHUNKS):
            base = sum(PIX_G[:g]) + s
            ash[:, j, :wdt] = at_full[i, base : base + wdt, :].T.astype(bf16)
        in_maps.append(
            {"xg": np.ascontiguousarray(row), "asb": np.ascontiguousarray(ash)}
        )
    return in_maps


def _run(in_maps, trace=False, **kwargs):
    from concourse import bass_utils

    nc = _build()
    return bass_utils.run_bass_kernel_spmd(
        nc, in_maps, list(range(N_CORES)), trace=trace, **kwargs
    )


def _combine(results, beta, A):
    t1 = 0.0
    t2 = 0.0
    for r in results:
        o = np.asarray(r["out"], dtype=np.float64)
        t1 += float(np.sum(o[0, : 2 * NG]))
        t2 += float(np.sum(o[0, 2 * NG : 3 * NG]))
    bf = np.asarray(beta, dtype=np.float64).reshape(KL, NB)
    af = np.asarray(A, dtype=np.float64).reshape(HW, NB)
    m = af.T @ af  # 3x3
    t3 = float(C) * SCALE * SCALE * float(np.einsum("kb,bc,kc->", bf, m, bf))
    total = t1 - 2.0 * SCALE * t2 + t3
    return np.float32(total / DENOM)


def kernel(x, beta, A):
    res = _run(_make_in_maps(x, beta, A))
    return _combine(res.results, beta, A)
